# revision 24
# baseline (speedup 1.0000x reference)
"""Trainium2 Bass kernel for batched multi-head attention (B=8, N=M=C=1024,
H=16, D=64), data-parallel across 8 NeuronCores (one batch element per core).

Per-core dataflow (bf16 matmul inputs, f32 PSUM accumulate):
  1. gpsimd cast-DMA q/k/v (f32) and target_mask (int32) to bf16 DRAM
     staging, then xbar DMA-transpose-load them into SBUF in transposed
     layout (contraction dims on partitions).
  2. Per head-pair j (c' slice of 128): project qh^T/kh^T (head-transposed)
     and vh (natural); key mask folded into vh scaling + a trailing
     "key-indicator" column that makes the AV matmul also produce the
     softmax denominator (no partition-dim reduction needed).
  3. Attention: QK^T as K=64 matmuls row-packed two heads at a time;
     exp on the scalar engine straight from PSUM with the 1/sqrt(D) scale
     folded in; target mask applied as one bf16 elementwise multiply on
     gpsimd; AV with 65-column lhsT -> numerator + denominator together.
  4. Normalize via a K=1 f32r ones-matmul broadcast of the denominator row
     + fast reciprocal + multiply; o-projection with bo folded in as a K=1
     ones matmul.
"""
import sys

sys.path.insert(0, "/opt/trn_rl_repo")

import numpy as np

import concourse.bass as bass  # noqa: F401
import concourse.mybir as mybir
import concourse.bacc as bacc
import concourse.tile as tile
from concourse import bass_utils

B = 8
N = 1024   # queries
M = 1024   # keys
C = 1024   # model dim
H = 16
D = 64
NP = 8     # head pairs
P = 128
NB = 2     # n blocks of 512
SCALE = D ** -0.5

F32 = mybir.dt.float32
F32R = mybir.dt.float32r
BF16 = mybir.dt.bfloat16
I32 = mybir.dt.int32
MUL = mybir.AluOpType.mult
EXP = mybir.ActivationFunctionType.Exp

_NC_CACHE = {}


def build_nc():
    nc = bacc.Bacc("TRN2", target_bir_lowering=False, debug=False, num_devices=1)

    q_d = nc.dram_tensor("q", [N, C], F32, kind="ExternalInput").ap()
    k_d = nc.dram_tensor("k", [M, C], F32, kind="ExternalInput").ap()
    v_d = nc.dram_tensor("v", [M, C], F32, kind="ExternalInput").ap()
    mask_d = nc.dram_tensor("mask", [M], I32, kind="ExternalInput").ap()
    tm_d = nc.dram_tensor("target_mask", [N, M], I32, kind="ExternalInput").ap()
    wq_d = nc.dram_tensor("Wq", [C, C], F32, kind="ExternalInput").ap()
    wk_d = nc.dram_tensor("Wk", [C, C], F32, kind="ExternalInput").ap()
    wv_d = nc.dram_tensor("Wv", [C, C], F32, kind="ExternalInput").ap()
    wo_d = nc.dram_tensor("Wo", [C, C], F32, kind="ExternalInput").ap()
    bo_d = nc.dram_tensor("bo", [C], F32, kind="ExternalInput").ap()
    out_d = nc.dram_tensor("out", [N, C], F32, kind="ExternalOutput").ap()

    with tile.TileContext(nc) as tc:
        _body(tc, nc, q_d, k_d, v_d, mask_d, tm_d, wq_d, wk_d, wv_d, wo_d,
              bo_d, out_d)
    nc.compile()
    return nc


def _body(tc, nc, q_d, k_d, v_d, mask_d, tm_d, wq_d, wk_d, wv_d, wo_d,
          bo_d, out_d):
    from contextlib import ExitStack
    ctx = ExitStack()
    with ctx:
        persist = ctx.enter_context(tc.tile_pool(name="persist", bufs=1))
        wpool = ctx.enter_context(tc.tile_pool(name="wpool", bufs=2))
        xpool = ctx.enter_context(tc.tile_pool(name="xpool", bufs=2))
        opool = ctx.enter_context(tc.tile_pool(name="opool", bufs=2))
        wvpool = ctx.enter_context(tc.tile_pool(name="wvpool", bufs=1))
        spsum = ctx.enter_context(tc.tile_pool(name="spsum", bufs=2, space="PSUM"))
        avpsum = ctx.enter_context(tc.tile_pool(name="avpsum", bufs=2, space="PSUM"))
        pjpsum = ctx.enter_context(tc.tile_pool(name="pjpsum", bufs=2, space="PSUM"))

        # ---- Phase 0/1: on-chip transposes. Contiguous row-chunk loads,
        # DVE bf16 cast, then PE transpose-mode into PSUM and a strided
        # copy out. No DRAM round trip, and the PE transposes warm up HAM.
        from concourse.masks import make_identity
        lpool = ctx.enter_context(tc.tile_pool(name="lpool", bufs=8))
        ident = persist.tile([P, P], BF16)
        make_identity(nc, ident[:])

        qbT = persist.tile([P, 8, N], BF16)   # [p, cc, n] = q[n, cc*128+p]
        kbT = persist.tile([P, 8, M], BF16)
        vbT = persist.tile([P, 8, M], BF16)
        tmT = persist.tile([P, 8, N], BF16)   # [p, mc, n] = tmask[n, mc*128+p]

        def stage_matrix(src_d, dstT, dt_in):
            src_re = src_d.rearrange("(rc p) c -> p rc c", p=P)
            for rc in range(8):
                for cg in range(2):
                    xb = lpool.tile([P, 512], BF16, tag="ldb")
                    nc.gpsimd.dma_start(out=xb[:],
                                        in_=src_re[:, rc, cg * 512:(cg + 1) * 512])
                    tp = spsum.tile([P, 4, P], BF16, tag="sp")
                    for ci in range(4):
                        nc.tensor.transpose(tp[:, ci, :],
                                            xb[:, ci * P:(ci + 1) * P], ident[:])
                    nc.vector.tensor_copy(
                        dstT[:, cg * 4:(cg + 1) * 4, rc * P:(rc + 1) * P], tp[:])

        stage_matrix(q_d, qbT, F32)
        stage_matrix(k_d, kbT, F32)
        stage_matrix(v_d, vbT, F32)
        stage_matrix(tm_d, tmT, I32)

        # ---- small constants ----
        mi = persist.tile([P, 8], I32)
        nc.sync.dma_start(out=mi[:], in_=mask_d.rearrange("(mc p) -> p mc", p=P))
        keyf = persist.tile([P, 8], F32)
        nc.vector.tensor_copy(keyf[:], mi[:])
        keyb = persist.tile([P, 8], BF16)
        nc.vector.tensor_copy(keyb[:], keyf[:])

        bo_f = persist.tile([1, C], F32)
        nc.sync.dma_start(out=bo_f[:], in_=bo_d.rearrange("(a c) -> a c", a=1))
        bob = persist.tile([1, C], BF16)
        nc.vector.tensor_copy(bob[:], bo_f[:])

        ones_f = persist.tile([P, D], F32)
        nc.vector.memset(ones_f[:], 1.0)
        onesr = persist.tile([P, D], F32R)
        nc.vector.tensor_copy(onesr[:], ones_f[:])
        onesb = persist.tile([1, P], BF16)
        nc.vector.memset(onesb[:], 1.0)

        qhT = persist.tile([P, NP, N], BF16)  # [p, j, n] = qh[n, j*128+p]
        khT = persist.tile([P, NP, M], BF16)
        vha = persist.tile([P, NP, 8, 130], BF16)
        xn = persist.tile([P, NP, N], BF16)   # [p, j, n] = x_norm[n, j*128+p]
        wob = persist.tile([P, NP, C], BF16)  # [p, j, c2] = Wo[j*128+p, c2]

        wq_re = wq_d.rearrange("(cc p) c2 -> p cc c2", p=P)
        wk_re = wk_d.rearrange("(cc p) c2 -> p cc c2", p=P)
        wv_re = wv_d.rearrange("(cc p) c2 -> p cc c2", p=P)
        wo_re = wo_d.rearrange("(j p) c2 -> p j c2", p=P)

        # ---- Phases 2+3 per head pair ----
        for j in range(NP):
            cs = slice(j * P, (j + 1) * P)
            # Wo slice for the o-projection tail, spread across pairs
            wo_f = wpool.tile([P, 8, P], F32, tag="wf")
            nc.sync.dma_start(out=wo_f[:].rearrange("p a b -> p (a b)"),
                              in_=wo_re[:, j, :])
            nc.scalar.copy(wob[:, j, :], wo_f[:].rearrange("p a b -> p (a b)"))
            wq_f = wpool.tile([P, 8, P], F32, tag="wf")
            nc.sync.dma_start(out=wq_f[:], in_=wq_re[:, :, cs])
            wqb = wpool.tile([P, 8, P], BF16, tag="wqb")
            nc.vector.tensor_copy(wqb[:], wq_f[:])
            wk_f = wpool.tile([P, 8, P], F32, tag="wf")
            nc.sync.dma_start(out=wk_f[:], in_=wk_re[:, :, cs])
            wkb = wpool.tile([P, 8, P], BF16, tag="wkb")
            nc.vector.tensor_copy(wkb[:], wk_f[:])
            if j % 2 == 0:
                wv_f = wvpool.tile([P, 8, 256], F32, tag="wvf")
                nc.sync.dma_start(out=wv_f[:], in_=wv_re[:, :, j * P:(j + 2) * P])
                wvb = wvpool.tile([P, 8, 256], BF16, tag="wvb")
                nc.vector.tensor_copy(wvb[:], wv_f[:])

            # q/k projections -> qhT/khT (head-transposed layout)
            for nb in range(NB):
                ns = slice(nb * 512, (nb + 1) * 512)
                pq = pjpsum.tile([P, 512], F32, tag="pj")
                for cc in range(8):
                    nc.tensor.matmul(pq[:], wqb[:, cc, :], qbT[:, cc, ns],
                                     start=(cc == 0), stop=(cc == 7))
                nc.scalar.copy(qhT[:, j, ns], pq[:])
                pk = pjpsum.tile([P, 512], F32, tag="pj")
                for cc in range(8):
                    nc.tensor.matmul(pk[:], wkb[:, cc, :], kbT[:, cc, ns],
                                     start=(cc == 0), stop=(cc == 7))
                nc.scalar.copy(khT[:, j, ns], pk[:])

            # v projection -> vha for pairs (j, j+1), 256-col matmuls
            if j % 2 == 0:
                for mc in range(8):
                    pv = pjpsum.tile([P, 256], F32, tag="pj")
                    for cc in range(8):
                        nc.tensor.matmul(pv[:], vbT[:, cc, mc * P:(mc + 1) * P],
                                         wvb[:, cc, :],
                                         start=(cc == 0), stop=(cc == 7))
                    out_sl = vha[:, j:j + 2, mc, :].rearrange(
                        "p j (hx dd) -> p j hx dd", hx=2)[:, :, :, 0:64]
                    in_sl = pv[:].rearrange("p (j hx dd) -> p j hx dd", j=2, hx=2)
                    nc.vector.tensor_scalar(out_sl, in_sl, keyf[:, mc:mc + 1], None,
                                            op0=MUL)
                for jx in (j, j + 1):
                    nc.vector.tensor_copy(vha[:, jx, :, 64], keyb[:])
                    nc.vector.tensor_copy(vha[:, jx, :, 129], keyb[:])

            # attention for the two heads of this pair
            for nb in range(NB):
                ns = slice(nb * 512, (nb + 1) * 512)
                ptiles = [[None] * 4, [None] * 4]
                for g in range(4):
                    sp0 = spsum.tile([P, 2, 512], F32, tag="sp")
                    sp1 = spsum.tile([P, 2, 512], F32, tag="sp")
                    for mcx in range(2):
                        mc = 2 * g + mcx
                        ms = slice(mc * P, (mc + 1) * P)
                        nc.tensor.matmul(sp0[:, mcx, :], khT[0:64, j, ms],
                                         qhT[0:64, j, ns], start=True, stop=True)
                        nc.tensor.matmul(sp1[:, mcx, :], khT[64:128, j, ms],
                                         qhT[64:128, j, ns], start=True, stop=True)
                    pt0 = lpool.tile([P, 2, 512], BF16, tag="ldb")
                    pt1 = lpool.tile([P, 2, 512], BF16, tag="ldb")
                    nc.scalar.activation(pt0[:], sp0[:], EXP, scale=SCALE)
                    nc.scalar.activation(pt1[:], sp1[:], EXP, scale=SCALE)
                    for mcx in range(2):
                        mc = 2 * g + mcx
                        tsl = tmT[:, mc, ns]
                        nc.vector.tensor_tensor(pt0[:, mcx, :], pt0[:, mcx, :], tsl, MUL)
                        nc.vector.tensor_tensor(pt1[:, mcx, :], pt1[:, mcx, :], tsl, MUL)
                    ptiles[0][g] = pt0
                    ptiles[1][g] = pt1

                av0 = avpsum.tile([65, 512], F32, tag="av")
                av1 = avpsum.tile([65, 512], F32, tag="av")
                for mc in range(8):
                    g, mcx = mc // 2, mc % 2
                    nc.tensor.matmul(av0[:], vha[:, j, mc, 0:65],
                                     ptiles[0][g][:, mcx, :],
                                     start=(mc == 0), stop=(mc == 7))
                    nc.tensor.matmul(av1[:], vha[:, j, mc, 65:130],
                                     ptiles[1][g][:, mcx, :],
                                     start=(mc == 0), stop=(mc == 7))

                # normalize: den row 64 -> broadcast (K=1 f32r matmul) ->
                # reciprocal -> multiply into xn
                for hx, av in ((0, av0), (1, av1)):
                    xu = xpool.tile([65, 512], F32R, tag="xu")
                    nc.vector.tensor_copy(xu[:], av[:])
                    bc = avpsum.tile([64, 512], F32, tag="av")
                    nc.tensor.matmul(bc[:], onesr[64:65, :], xu[64:65, :],
                                     start=True, stop=True)
                    rc = xpool.tile([64, 512], F32, tag="rc")
                    nc.vector.reciprocal_approx_fast(rc[:], bc[:])
                    rows = slice(0, 64) if hx == 0 else slice(64, 128)
                    nc.vector.tensor_tensor(xn[rows, j, ns],
                                            xu[0:64, :].bitcast(F32), rc[:], MUL)

        # ---- Phase 4: o-projection (+ bias) ----
        # Even/odd head rows are row-packed (disjoint PE row groups run
        # concurrently), so they must accumulate into *separate* PSUM banks;
        # the two partial sums are combined in the copy-out.
        for nch in range(8):
            nsl = slice(nch * P, (nch + 1) * P)
            for c2h in range(2):
                c2s = slice(c2h * 512, (c2h + 1) * 512)
                po = pjpsum.tile([P, 512], F32, tag="pj")
                nc.tensor.matmul(po[:], onesb[0:1, :], bob[0:1, c2s],
                                 start=True, stop=False)
                for j in range(NP):
                    nc.tensor.matmul(po[:], xn[:, j, nsl], wob[:, j, c2s],
                                     start=False, stop=(j == NP - 1))
                ot = opool.tile([P, 512], F32, tag="ot")
                nc.scalar.copy(ot[:], po[:])
                nc.sync.dma_start(out=out_d[nsl, c2s], in_=ot[:])


def _get_nc():
    if "nc" not in _NC_CACHE:
        _NC_CACHE["nc"] = build_nc()
    return _NC_CACHE["nc"]


def kernel(q, k, v, mask, target_mask, Wq, Wk, Wv, Wo, bo):
    nc = _get_nc()
    q = np.ascontiguousarray(np.asarray(q, np.float32))
    k = np.ascontiguousarray(np.asarray(k, np.float32))
    v = np.ascontiguousarray(np.asarray(v, np.float32))
    mask = np.ascontiguousarray(np.asarray(mask, np.int32))
    target_mask = np.ascontiguousarray(np.asarray(target_mask, np.int32))
    shared = {
        "Wq": np.ascontiguousarray(np.asarray(Wq, np.float32)),
        "Wk": np.ascontiguousarray(np.asarray(Wk, np.float32)),
        "Wv": np.ascontiguousarray(np.asarray(Wv, np.float32)),
        "Wo": np.ascontiguousarray(np.asarray(Wo, np.float32)),
        "bo": np.ascontiguousarray(np.asarray(bo, np.float32)),
    }
    in_maps = []
    for b in range(B):
        m = {"q": q[b], "k": k[b], "v": v[b], "mask": mask[b],
             "target_mask": target_mask[b]}
        m.update(shared)
        in_maps.append(m)
    res = bass_utils.run_bass_kernel_spmd(nc, in_maps, core_ids=list(range(B)))
    out = np.stack([res.results[b]["out"] for b in range(B)], axis=0)
    return out.astype(np.float32)


def run_traced(q, k, v, mask, target_mask, Wq, Wk, Wv, Wo, bo, **trace_kwargs):
    """Like kernel() but with NTFF tracing; returns (out, BassKernelResults)."""
    nc = _get_nc()
    in_maps = []
    for b in range(B):
        m = {"q": np.asarray(q[b], np.float32), "k": np.asarray(k[b], np.float32),
             "v": np.asarray(v[b], np.float32),
             "mask": np.asarray(mask[b], np.int32),
             "target_mask": np.asarray(target_mask[b], np.int32),
             "Wq": np.asarray(Wq, np.float32), "Wk": np.asarray(Wk, np.float32),
             "Wv": np.asarray(Wv, np.float32), "Wo": np.asarray(Wo, np.float32),
             "bo": np.asarray(bo, np.float32)}
        in_maps.append(m)
    res = bass_utils.run_bass_kernel_spmd(nc, in_maps, core_ids=list(range(B)),
                                          trace=True, **trace_kwargs)
    out = np.stack([res.results[b]["out"] for b in range(B)], axis=0)
    return out.astype(np.float32), res


# revision 25
# speedup vs baseline: 1.0179x; 1.0179x over previous
"""Trainium2 Bass kernel for batched multi-head attention (B=8, N=M=C=1024,
H=16, D=64), data-parallel across 8 NeuronCores (one batch element per core).

Per-core dataflow (bf16 matmul inputs, f32 PSUM accumulate):
  1. Staging: gpsimd SWDGE cast-loads (f32/int32 -> bf16 straight into
     SBUF), then PE transpose-mode matmuls put q/k/v/target_mask into
     transposed layout (contraction dims on partitions). No DRAM round
     trip; the transposes also warm up the PE clock gate.
  2. Per head-pair j (c' slice of 128): project qh^T/kh^T (head-transposed)
     and vh (natural); key mask folded into vh scaling + a trailing
     "key-indicator" column that makes the AV matmul also produce the
     softmax denominator (no partition-dim reduction needed).
  3. Attention: QK^T as K=64 matmuls row-packed two heads at a time
     (disjoint PE row groups run concurrently, separate PSUM banks);
     exp on the scalar engine straight from PSUM with the 1/sqrt(D)
     scale folded in; target mask as contiguous [128,512] bf16 DVE
     multiplies (2x mode); AV with 65-column lhsT -> numerator +
     denominator in one accumulation chain.
  4. Normalize via a K=1 f32r ones-matmul broadcast of the denominator row
     + fast reciprocal + multiply; o-projection as K=128 accumulation
     chains with bo folded in as a K=1 ones matmul.
"""
import sys

sys.path.insert(0, "/opt/trn_rl_repo")

import numpy as np

import concourse.bass as bass  # noqa: F401
import concourse.mybir as mybir
import concourse.bacc as bacc
import concourse.tile as tile
from concourse import bass_utils

B = 8
N = 1024   # queries
M = 1024   # keys
C = 1024   # model dim
H = 16
D = 64
NP = 8     # head pairs
P = 128
NB = 2     # n blocks of 512
SCALE = D ** -0.5

F32 = mybir.dt.float32
F32R = mybir.dt.float32r
BF16 = mybir.dt.bfloat16
I32 = mybir.dt.int32
MUL = mybir.AluOpType.mult
EXP = mybir.ActivationFunctionType.Exp

_NC_CACHE = {}


def build_nc():
    nc = bacc.Bacc("TRN2", target_bir_lowering=False, debug=False, num_devices=1)

    q_d = nc.dram_tensor("q", [N, C], F32, kind="ExternalInput").ap()
    k_d = nc.dram_tensor("k", [M, C], F32, kind="ExternalInput").ap()
    v_d = nc.dram_tensor("v", [M, C], F32, kind="ExternalInput").ap()
    mask_d = nc.dram_tensor("mask", [M], I32, kind="ExternalInput").ap()
    tm_d = nc.dram_tensor("target_mask", [N, M], I32, kind="ExternalInput").ap()
    wq_d = nc.dram_tensor("Wq", [C, C], F32, kind="ExternalInput").ap()
    wk_d = nc.dram_tensor("Wk", [C, C], F32, kind="ExternalInput").ap()
    wv_d = nc.dram_tensor("Wv", [C, C], F32, kind="ExternalInput").ap()
    wo_d = nc.dram_tensor("Wo", [C, C], F32, kind="ExternalInput").ap()
    bo_d = nc.dram_tensor("bo", [C], F32, kind="ExternalInput").ap()
    out_d = nc.dram_tensor("out", [N, C], F32, kind="ExternalOutput").ap()

    with tile.TileContext(nc) as tc:
        _body(tc, nc, q_d, k_d, v_d, mask_d, tm_d, wq_d, wk_d, wv_d, wo_d,
              bo_d, out_d)
    nc.compile()
    return nc


def _body(tc, nc, q_d, k_d, v_d, mask_d, tm_d, wq_d, wk_d, wv_d, wo_d,
          bo_d, out_d):
    from contextlib import ExitStack
    ctx = ExitStack()
    with ctx:
        persist = ctx.enter_context(tc.tile_pool(name="persist", bufs=1))
        wpool = ctx.enter_context(tc.tile_pool(name="wpool", bufs=2))
        xpool = ctx.enter_context(tc.tile_pool(name="xpool", bufs=2))
        opool = ctx.enter_context(tc.tile_pool(name="opool", bufs=2))
        wvpool = ctx.enter_context(tc.tile_pool(name="wvpool", bufs=1))
        spsum = ctx.enter_context(tc.tile_pool(name="spsum", bufs=2, space="PSUM"))
        avpsum = ctx.enter_context(tc.tile_pool(name="avpsum", bufs=2, space="PSUM"))
        pjpsum = ctx.enter_context(tc.tile_pool(name="pjpsum", bufs=2, space="PSUM"))

        # ---- Phase 0/1: on-chip transposes. Contiguous row-chunk loads,
        # DVE bf16 cast, then PE transpose-mode into PSUM and a strided
        # copy out. No DRAM round trip, and the PE transposes warm up HAM.
        from concourse.masks import make_identity
        lpool = ctx.enter_context(tc.tile_pool(name="lpool", bufs=8))
        ident = persist.tile([P, P], BF16)
        make_identity(nc, ident[:])

        qbT = persist.tile([P, 8, N], BF16)   # [p, cc, n] = q[n, cc*128+p]
        kbT = persist.tile([P, 8, M], BF16)
        vbT = persist.tile([P, 8, M], BF16)
        tmT = persist.tile([P, 8, N], BF16)   # [p, mc, n] = tmask[n, mc*128+p]

        def stage_matrix(src_d, dstT, dt_in):
            src_re = src_d.rearrange("(rc p) c -> p rc c", p=P)
            for rc in range(8):
                for cg in range(2):
                    xb = lpool.tile([P, 512], BF16, tag="ldb")
                    nc.gpsimd.dma_start(out=xb[:],
                                        in_=src_re[:, rc, cg * 512:(cg + 1) * 512])
                    tp = spsum.tile([P, 4, P], BF16, tag="sp")
                    for ci in range(4):
                        nc.tensor.transpose(tp[:, ci, :],
                                            xb[:, ci * P:(ci + 1) * P], ident[:])
                    nc.vector.tensor_copy(
                        dstT[:, cg * 4:(cg + 1) * 4, rc * P:(rc + 1) * P], tp[:])

        stage_matrix(q_d, qbT, F32)
        stage_matrix(k_d, kbT, F32)
        stage_matrix(v_d, vbT, F32)
        stage_matrix(tm_d, tmT, I32)

        # ---- small constants ----
        mi = persist.tile([P, 8], I32)
        nc.sync.dma_start(out=mi[:], in_=mask_d.rearrange("(mc p) -> p mc", p=P))
        keyf = persist.tile([P, 8], F32)
        nc.vector.tensor_copy(keyf[:], mi[:])
        keyb = persist.tile([P, 8], BF16)
        nc.vector.tensor_copy(keyb[:], keyf[:])

        bo_f = persist.tile([1, C], F32)
        nc.sync.dma_start(out=bo_f[:], in_=bo_d.rearrange("(a c) -> a c", a=1))
        bob = persist.tile([1, C], BF16)
        nc.vector.tensor_copy(bob[:], bo_f[:])

        ones_f = persist.tile([P, D], F32)
        nc.vector.memset(ones_f[:], 1.0)
        onesr = persist.tile([P, D], F32R)
        nc.vector.tensor_copy(onesr[:], ones_f[:])
        onesb = persist.tile([1, P], BF16)
        nc.vector.memset(onesb[:], 1.0)

        qhT = persist.tile([P, NP, N], BF16)  # [p, j, n] = qh[n, j*128+p]
        khT = persist.tile([P, NP, M], BF16)
        vha = persist.tile([P, NP, 8, 130], BF16)
        xn = persist.tile([P, NP, N], BF16)   # [p, j, n] = x_norm[n, j*128+p]
        wob = persist.tile([P, NP, C], BF16)  # [p, j, c2] = Wo[j*128+p, c2]

        wq_re = wq_d.rearrange("(cc p) c2 -> p cc c2", p=P)
        wk_re = wk_d.rearrange("(cc p) c2 -> p cc c2", p=P)
        wv_re = wv_d.rearrange("(cc p) c2 -> p cc c2", p=P)
        wo_re = wo_d.rearrange("(j p) c2 -> p j c2", p=P)

        # ---- Phases 2+3 per head pair ----
        for j in range(NP):
            cs = slice(j * P, (j + 1) * P)
            # Wo slice for the o-projection tail, spread across pairs
            wo_f = wpool.tile([P, 8, P], F32, tag="wf")
            nc.sync.dma_start(out=wo_f[:].rearrange("p a b -> p (a b)"),
                              in_=wo_re[:, j, :])
            nc.scalar.copy(wob[:, j, :], wo_f[:].rearrange("p a b -> p (a b)"))
            wq_f = wpool.tile([P, 8, P], F32, tag="wf")
            nc.sync.dma_start(out=wq_f[:], in_=wq_re[:, :, cs])
            wqb = wpool.tile([P, 8, P], BF16, tag="wqb")
            nc.vector.tensor_copy(wqb[:], wq_f[:])
            wk_f = wpool.tile([P, 8, P], F32, tag="wf")
            nc.sync.dma_start(out=wk_f[:], in_=wk_re[:, :, cs])
            wkb = wpool.tile([P, 8, P], BF16, tag="wkb")
            nc.vector.tensor_copy(wkb[:], wk_f[:])
            if j % 2 == 0:
                wv_f = wvpool.tile([P, 8, 256], F32, tag="wvf")
                nc.sync.dma_start(out=wv_f[:], in_=wv_re[:, :, j * P:(j + 2) * P])
                wvb = wvpool.tile([P, 8, 256], BF16, tag="wvb")
                nc.vector.tensor_copy(wvb[:], wv_f[:])

            # q/k projections -> qhT/khT (head-transposed layout)
            for nb in range(NB):
                ns = slice(nb * 512, (nb + 1) * 512)
                pq = pjpsum.tile([P, 512], F32, tag="pj")
                for cc in range(8):
                    nc.tensor.matmul(pq[:], wqb[:, cc, :], qbT[:, cc, ns],
                                     start=(cc == 0), stop=(cc == 7))
                nc.scalar.copy(qhT[:, j, ns], pq[:])
                pk = pjpsum.tile([P, 512], F32, tag="pj")
                for cc in range(8):
                    nc.tensor.matmul(pk[:], wkb[:, cc, :], kbT[:, cc, ns],
                                     start=(cc == 0), stop=(cc == 7))
                nc.scalar.copy(khT[:, j, ns], pk[:])

            # v projection -> vha for pairs (j, j+1), 256-col matmuls
            if j % 2 == 0:
                for mc in range(8):
                    pv = pjpsum.tile([P, 256], F32, tag="pj")
                    for cc in range(8):
                        nc.tensor.matmul(pv[:], vbT[:, cc, mc * P:(mc + 1) * P],
                                         wvb[:, cc, :],
                                         start=(cc == 0), stop=(cc == 7))
                    out_sl = vha[:, j:j + 2, mc, :].rearrange(
                        "p j (hx dd) -> p j hx dd", hx=2)[:, :, :, 0:64]
                    in_sl = pv[:].rearrange("p (j hx dd) -> p j hx dd", j=2, hx=2)
                    nc.vector.tensor_scalar(out_sl, in_sl, keyf[:, mc:mc + 1], None,
                                            op0=MUL)
                for jx in (j, j + 1):
                    nc.vector.tensor_copy(vha[:, jx, :, 64], keyb[:])
                    nc.vector.tensor_copy(vha[:, jx, :, 129], keyb[:])

            # attention for the two heads of this pair
            for nb in range(NB):
                ns = slice(nb * 512, (nb + 1) * 512)
                ptiles = [[None] * 4, [None] * 4]
                for g in range(4):
                    sp0 = spsum.tile([P, 2, 512], F32, tag="sp")
                    sp1 = spsum.tile([P, 2, 512], F32, tag="sp")
                    for mcx in range(2):
                        mc = 2 * g + mcx
                        ms = slice(mc * P, (mc + 1) * P)
                        nc.tensor.matmul(sp0[:, mcx, :], khT[0:64, j, ms],
                                         qhT[0:64, j, ns], start=True, stop=True)
                        nc.tensor.matmul(sp1[:, mcx, :], khT[64:128, j, ms],
                                         qhT[64:128, j, ns], start=True, stop=True)
                    pt0 = lpool.tile([P, 2, 512], BF16, tag="ldb")
                    pt1 = lpool.tile([P, 2, 512], BF16, tag="ldb")
                    nc.scalar.activation(pt0[:], sp0[:], EXP, scale=SCALE)
                    nc.scalar.activation(pt1[:], sp1[:], EXP, scale=SCALE)
                    for mcx in range(2):
                        mc = 2 * g + mcx
                        tsl = tmT[:, mc, ns]
                        nc.vector.tensor_tensor(pt0[:, mcx, :], pt0[:, mcx, :], tsl, MUL)
                        nc.vector.tensor_tensor(pt1[:, mcx, :], pt1[:, mcx, :], tsl, MUL)
                    ptiles[0][g] = pt0
                    ptiles[1][g] = pt1

                av0 = avpsum.tile([65, 512], F32, tag="av")
                av1 = avpsum.tile([65, 512], F32, tag="av")
                for mc in range(8):
                    g, mcx = mc // 2, mc % 2
                    nc.tensor.matmul(av0[:], vha[:, j, mc, 0:65],
                                     ptiles[0][g][:, mcx, :],
                                     start=(mc == 0), stop=(mc == 7))
                    nc.tensor.matmul(av1[:], vha[:, j, mc, 65:130],
                                     ptiles[1][g][:, mcx, :],
                                     start=(mc == 0), stop=(mc == 7))

                # normalize: den row 64 -> broadcast (K=1 f32r matmul) ->
                # reciprocal -> multiply into xn
                for hx, av in ((0, av0), (1, av1)):
                    xu = xpool.tile([65, 512], F32R, tag="xu")
                    nc.vector.tensor_copy(xu[:], av[:])
                    bc = avpsum.tile([64, 512], F32, tag="av")
                    nc.tensor.matmul(bc[:], onesr[64:65, :], xu[64:65, :],
                                     start=True, stop=True)
                    rc = xpool.tile([64, 512], F32, tag="rc")
                    nc.vector.reciprocal_approx_fast(rc[:], bc[:])
                    rows = slice(0, 64) if hx == 0 else slice(64, 128)
                    nc.vector.tensor_tensor(xn[rows, j, ns],
                                            xu[0:64, :].bitcast(F32), rc[:], MUL)

        # ---- Phase 4: o-projection (+ bias) ----
        # Even/odd head rows are row-packed (disjoint PE row groups run
        # concurrently), so they must accumulate into *separate* PSUM banks;
        # the two partial sums are combined in the copy-out.
        for nch in range(8):
            nsl = slice(nch * P, (nch + 1) * P)
            for c2h in range(2):
                c2s = slice(c2h * 512, (c2h + 1) * 512)
                po = pjpsum.tile([P, 512], F32, tag="pj")
                nc.tensor.matmul(po[:], onesb[0:1, :], bob[0:1, c2s],
                                 start=True, stop=False)
                for j in range(NP):
                    nc.tensor.matmul(po[:], xn[:, j, nsl], wob[:, j, c2s],
                                     start=False, stop=(j == NP - 1))
                ot = opool.tile([P, 512], F32, tag="ot")
                nc.scalar.copy(ot[:], po[:])
                nc.sync.dma_start(out=out_d[nsl, c2s], in_=ot[:])


def _get_nc():
    if "nc" not in _NC_CACHE:
        _NC_CACHE["nc"] = build_nc()
    return _NC_CACHE["nc"]


def kernel(q, k, v, mask, target_mask, Wq, Wk, Wv, Wo, bo):
    nc = _get_nc()
    q = np.ascontiguousarray(np.asarray(q, np.float32))
    k = np.ascontiguousarray(np.asarray(k, np.float32))
    v = np.ascontiguousarray(np.asarray(v, np.float32))
    mask = np.ascontiguousarray(np.asarray(mask, np.int32))
    target_mask = np.ascontiguousarray(np.asarray(target_mask, np.int32))
    shared = {
        "Wq": np.ascontiguousarray(np.asarray(Wq, np.float32)),
        "Wk": np.ascontiguousarray(np.asarray(Wk, np.float32)),
        "Wv": np.ascontiguousarray(np.asarray(Wv, np.float32)),
        "Wo": np.ascontiguousarray(np.asarray(Wo, np.float32)),
        "bo": np.ascontiguousarray(np.asarray(bo, np.float32)),
    }
    in_maps = []
    for b in range(B):
        m = {"q": q[b], "k": k[b], "v": v[b], "mask": mask[b],
             "target_mask": target_mask[b]}
        m.update(shared)
        in_maps.append(m)
    res = bass_utils.run_bass_kernel_spmd(nc, in_maps, core_ids=list(range(B)))
    out = np.stack([res.results[b]["out"] for b in range(B)], axis=0)
    return out.astype(np.float32)


def run_traced(q, k, v, mask, target_mask, Wq, Wk, Wv, Wo, bo, **trace_kwargs):
    """Like kernel() but with NTFF tracing; returns (out, BassKernelResults)."""
    nc = _get_nc()
    in_maps = []
    for b in range(B):
        m = {"q": np.asarray(q[b], np.float32), "k": np.asarray(k[b], np.float32),
             "v": np.asarray(v[b], np.float32),
             "mask": np.asarray(mask[b], np.int32),
             "target_mask": np.asarray(target_mask[b], np.int32),
             "Wq": np.asarray(Wq, np.float32), "Wk": np.asarray(Wk, np.float32),
             "Wv": np.asarray(Wv, np.float32), "Wo": np.asarray(Wo, np.float32),
             "bo": np.asarray(bo, np.float32)}
        in_maps.append(m)
    res = bass_utils.run_bass_kernel_spmd(nc, in_maps, core_ids=list(range(B)),
                                          trace=True, **trace_kwargs)
    out = np.stack([res.results[b]["out"] for b in range(B)], axis=0)
    return out.astype(np.float32), res
